# revision 2
# baseline (speedup 1.0000x reference)
"""Trainium2 Bass kernel for batched masked attention.

Problem: q,k,v [16, 2048, 256] f32, mask [16, 2048, 2048] int32.
  scores = (q @ k^T) / 16
  scores = where(mask == 0, 0.0, scores)      # NOT -inf
  att    = softmax(scores, axis=-1)
  att    = 0 if mask.sum() == 0 (handled host-side)
  out    = att @ v

Sharding: batch dim across 8 NeuronCores (2 batches per core); each core
computes full attention for its batches independently; host gathers.

The host pre-arranges inputs into the exact on-chip layouts (all free — the
kernel owns its input contract):
  qh/ql/kh/kl : [BPC, 128, 2, S] fp8e4 — hi/lo split of q/k, head-dim on
          partitions (transposed), the middle dim is the DoubleRow K-group
  vp    : [BPC, 128, S/128, D+2] bf16 — v tiles with two ones columns; in the
          output matmul the ones column accumulates the softmax denominator Z
  mask8 : [BPC, 4, 128, S/128, 512] u8 — mask transposed (key-major) and cast
          to u8, pre-tiled per 512-query chunk
Everything is computed in the transposed score domain so no on-chip
transposes are needed at all; per 512-query chunk:
  mm1 (PE, fp8 DoubleRow, 3-term hi/lo compensation — q@k to ~bf16 accuracy
      at 0.75x the bf16 instruction cost; each DR matmul contracts the full
      K=256 in one shot):
        sT[128 key, 512 qry] = kh.T qh + kh.T ql + kl.T qh
  DVE in-place:            sT = (sT * 1/16) * mask8           (u8 mask)
  ACT:                     attT = exp(sT)  PSUM->SBUF, bf16
  mm2 (PE, bf16):          out[128 qry, 258] += attT.T @ v'   (16 accums)
  DVE: 1/Z + scale-copy -> out tile (bf16) -> DMA; host upcasts to f32
mm2 for chunk ic-1 is emitted after mm1 of chunk ic (software pipelining) so
the PE never idles on the DVE/ACT epilogue; batch-level loads ride the
gpsimd SWDGE ring to keep the sync ring free for mask/out streaming.
"""

import sys

if "/opt/trn_rl_repo" not in sys.path:
    sys.path.insert(0, "/opt/trn_rl_repo")

from contextlib import ExitStack

import numpy as np
import ml_dtypes

import concourse.mybir as mybir
import concourse.tile as tile
from concourse import bacc
from concourse.bass_utils import run_bass_kernel_spmd

B, S, D = 16, 2048, 256
NCORES = 8
BPC = B // NCORES  # batches per core
P = 128
QT = S // P        # 16 key blocks of 128
IC = S // 512      # 4 query chunks of 512
KC = D // P        # 2 contraction chunks of 128 (the DoubleRow K-groups)
SCALE = 1.0 / 16.0  # 1/sqrt(D)

F32 = mybir.dt.float32
BF16 = mybir.dt.bfloat16
FP8 = mybir.dt.float8e4
U8 = mybir.dt.uint8
DR = mybir.MatmulPerfMode.DoubleRow


def build_program(reps=1):
    nc = bacc.Bacc("TRN2", target_bir_lowering=False, debug=False)
    qhd = nc.dram_tensor("qh", [BPC, P, KC, S], FP8, kind="ExternalInput").ap()
    qld = nc.dram_tensor("ql", [BPC, P, KC, S], FP8, kind="ExternalInput").ap()
    khd = nc.dram_tensor("kh", [BPC, P, KC, S], FP8, kind="ExternalInput").ap()
    kld = nc.dram_tensor("kl", [BPC, P, KC, S], FP8, kind="ExternalInput").ap()
    vpd = nc.dram_tensor("vp", [BPC, P, QT, D + 2], BF16, kind="ExternalInput").ap()
    m8d = nc.dram_tensor("mask8", [BPC, IC, P, QT, 512], U8, kind="ExternalInput").ap()
    out = nc.dram_tensor("out", [BPC, S, D], BF16, kind="ExternalOutput").ap()

    with tile.TileContext(nc) as tc, ExitStack() as ctx:
        kh_pool = ctx.enter_context(tc.tile_pool(name="kh", bufs=2))
        kl_pool = ctx.enter_context(tc.tile_pool(name="kl", bufs=2))
        qh_pool = ctx.enter_context(tc.tile_pool(name="qh", bufs=2))
        ql_pool = ctx.enter_context(tc.tile_pool(name="ql", bufs=2))
        vp_pool = ctx.enter_context(tc.tile_pool(name="vp", bufs=2))
        mask_pool = ctx.enter_context(tc.tile_pool(name="maskp", bufs=3))
        att_pool = ctx.enter_context(tc.tile_pool(name="att", bufs=2))
        osb_pool = ctx.enter_context(tc.tile_pool(name="osb", bufs=4))
        rec_pool = ctx.enter_context(tc.tile_pool(name="rec", bufs=4))
        # ps_s tiles span 2 PSUM banks (a PAIR of key blocks) so one DVE op
        # and one ACT exp cover 1024 columns, halving their per-op overhead
        ps_s = ctx.enter_context(tc.tile_pool(name="ps_s", bufs=3, space="PSUM"))
        ps_out = ctx.enter_context(tc.tile_pool(name="ps_out", bufs=2, space="PSUM"))

        def build_inputs(b):
            # chunked loads so each mm1 only waits for the slices it reads
            # (Tile tracks sub-tile AP ranges)
            kh = kh_pool.tile([P, KC, S], FP8, tag="kh")
            kl = kl_pool.tile([P, KC, S], FP8, tag="kl")
            qh = qh_pool.tile([P, KC, S], FP8, tag="qh")
            ql = ql_pool.tile([P, KC, S], FP8, tag="ql")
            nc.gpsimd.dma_start(qh[:, :, :512], qhd[b][:, :, :512])
            nc.gpsimd.dma_start(ql[:, :, :512], qld[b][:, :, :512])
            for jb in range(4):
                sl = slice(jb * P, (jb + 1) * P)
                nc.gpsimd.dma_start(kh[:, :, sl], khd[b][:, :, sl])
                nc.gpsimd.dma_start(kl[:, :, sl], kld[b][:, :, sl])
            for c in range(1, IC):
                sl = slice(c * 512, (c + 1) * 512)
                nc.gpsimd.dma_start(kh[:, :, sl], khd[b][:, :, sl])
                nc.gpsimd.dma_start(kl[:, :, sl], kld[b][:, :, sl])
            for c in range(1, IC):
                sl = slice(c * 512, (c + 1) * 512)
                nc.gpsimd.dma_start(qh[:, :, sl], qhd[b][:, :, sl])
                nc.gpsimd.dma_start(ql[:, :, sl], qld[b][:, :, sl])
            vp = vp_pool.tile([P, QT, D + 2], BF16, tag="vp")
            nc.gpsimd.dma_start(vp[:], vpd[b])
            return (kh, kl, qh, ql), vp

        def mm1_group(b, ic, g, kts, qts, mt, att):
            """scoresT + mask + exp for key blocks 4g..4g+3 of query chunk ic."""
            kh, kl, qh, ql = kts
            qsl = slice(ic * 512, (ic + 1) * 512)
            for jp in range(2 * g, 2 * g + 2):  # pairs of key blocks
                ps = ps_s.tile([P, 1024], F32, tag="score")
                for half in range(2):
                    jb = 2 * jp + half
                    ksl = slice(jb * P, (jb + 1) * P)
                    po = ps[:, half * 512 : (half + 1) * 512]
                    # 3-term compensated fp8 product; each DoubleRow matmul
                    # contracts both 128-dim groups (full K=256) at once
                    nc.tensor.matmul(
                        po, lhsT=kh[:, :, ksl], rhs=qts[0][:, :, qsl],
                        start=True, stop=False, perf_mode=DR,
                    )
                    nc.tensor.matmul(
                        po, lhsT=kh[:, :, ksl], rhs=qts[1][:, :, qsl],
                        start=False, stop=False, perf_mode=DR,
                    )
                    nc.tensor.matmul(
                        po, lhsT=kl[:, :, ksl], rhs=qts[0][:, :, qsl],
                        start=False, stop=True, perf_mode=DR,
                    )
                nc.vector.scalar_tensor_tensor(
                    out=ps[:],
                    in0=ps[:],
                    scalar=SCALE,
                    in1=mt[:, 2 * jp : 2 * jp + 2, :],
                    op0=mybir.AluOpType.mult,
                    op1=mybir.AluOpType.mult,
                )
                nc.scalar.activation(
                    att[:, 2 * jp : 2 * jp + 2, :],
                    ps[:],
                    mybir.ActivationFunctionType.Exp,
                )

        def mm2_group(b, ic, att, vp, iq):
            """att.T @ v' + normalize + store for query tile iq of chunk ic."""
            po = ps_out.tile([P, D + 2], F32, tag="ps_out")
            for jb in range(QT):
                nc.tensor.matmul(
                    po[:],
                    lhsT=att[:, jb, iq * P : (iq + 1) * P],
                    rhs=vp[:, jb, :],
                    start=(jb == 0),
                    stop=(jb == QT - 1),
                )
            rec = rec_pool.tile([P, 1], F32, tag="rec")
            nc.vector.reciprocal(rec[:], po[:, D : D + 1])
            osb = osb_pool.tile([P, D], BF16, tag="osb")
            nc.scalar.activation(
                osb[:],
                po[:, :D],
                mybir.ActivationFunctionType.Copy,
                scale=rec[:],
            )
            it = ic * 4 + iq
            nc.sync.dma_start(out[b, it * P : (it + 1) * P, :], osb[:])

        # Software-pipelined emission: mm2 groups for chunk ic-1 interleave
        # with mm1 groups for chunk ic, so the PE never waits on the DVE/ACT
        # epilogue; next batch's loads are emitted mid-batch for prefetch.
        batches = [b for _ in range(reps) for b in range(BPC)]
        # PE warm-up: ~4us of dummy matmuls during the initial DMA wait so
        # the HAM clock gate is at 2.4 GHz when real work arrives.
        warm = mask_pool.tile([P, 512], F32, tag="warm")
        nc.gpsimd.memset(warm[:], 0.0)
        for i in range(2):
            wp = ps_out.tile([P, 512], F32, tag="ps_out")
            nc.tensor.matmul(
                wp[:], lhsT=warm[:, :P], rhs=warm[:], start=True, stop=True
            )
        inputs = {0: build_inputs(batches[0])}
        pending = None
        for idx, b in enumerate(batches):
            kts_vp = inputs.pop(idx)
            kts, vp = kts_vp[0], kts_vp[1]
            qts = (kts[2], kts[3])  # qh, ql
            kmats = (kts[0], kts[1], kts[2], kts[3])
            for ic in range(IC):
                mt = mask_pool.tile([P, QT, 512], U8, tag="maskt")
                if idx == 0 and ic == 0:
                    # split the first mask load so STT on key block 0 starts
                    # after 256KB instead of 1MB
                    for g4 in range(4):
                        nc.sync.dma_start(
                            mt[:, g4 * 4 : (g4 + 1) * 4, :],
                            m8d[b, ic, :, g4 * 4 : (g4 + 1) * 4, :],
                        )
                else:
                    nc.sync.dma_start(mt[:], m8d[b, ic])
                att = att_pool.tile([P, QT, 512], BF16, tag="att")
                for g in range(4):
                    mm1_group(b, ic, g, kmats, qts, mt, att)
                    if pending is not None:
                        mm2_group(*pending, iq=g)
                if ic == 1 and idx + 1 < len(batches):
                    inputs[idx + 1] = build_inputs(batches[idx + 1])
                pending = (b, ic, att, vp)
        for g in range(4):
            mm2_group(*pending, iq=g)

    nc.compile()
    return nc


def prep_inputs(q, k, v, mask):
    """Host-side layout prep; returns per-core in_maps."""
    q = np.asarray(q, dtype=np.float32)
    k = np.asarray(k, dtype=np.float32)
    v = np.asarray(v, dtype=np.float32)
    FP8NP = ml_dtypes.float8_e4m3
    BF16NP = ml_dtypes.bfloat16
    # [B, S, D] -> [B, P, KC, S]  (transposed, head-dim on partitions)
    qt = np.ascontiguousarray(
        q.transpose(0, 2, 1).reshape(B, KC, P, S).transpose(0, 2, 1, 3)
    )
    kt = np.ascontiguousarray(
        k.transpose(0, 2, 1).reshape(B, KC, P, S).transpose(0, 2, 1, 3)
    )
    # hi/lo fp8 split: x = hi + lo to ~0.1% relative accuracy
    qh = qt.astype(FP8NP)
    ql = (qt - qh.astype(np.float32)).astype(FP8NP)
    kh = kt.astype(FP8NP)
    kl = (kt - kh.astype(np.float32)).astype(FP8NP)
    # [B, S, D] -> [B, P, QT, D+2] with ones in the last two columns
    vp = np.ones((B, P, QT, D + 2), dtype=BF16NP)
    vp[..., :D] = v.reshape(B, QT, P, D).transpose(0, 2, 1, 3).astype(BF16NP)
    # mask [B, S(query), S(key)] -> u8 tiles [B, IC, P(key), QT, 512(query)]
    m8 = np.ascontiguousarray(
        (np.asarray(mask) != 0)
        .astype(np.uint8)
        .reshape(B, IC, 512, QT, P)
        .transpose(0, 1, 4, 3, 2)
    )
    return [
        {
            "qh": qh[c * BPC : (c + 1) * BPC],
            "ql": ql[c * BPC : (c + 1) * BPC],
            "kh": kh[c * BPC : (c + 1) * BPC],
            "kl": kl[c * BPC : (c + 1) * BPC],
            "vp": vp[c * BPC : (c + 1) * BPC],
            "mask8": m8[c * BPC : (c + 1) * BPC],
        }
        for c in range(NCORES)
    ]


_NC_CACHE = None


def _get_program():
    global _NC_CACHE
    if _NC_CACHE is None:
        _NC_CACHE = build_program()
    return _NC_CACHE


def kernel(q, k, v, mask):
    mask = np.asarray(mask)
    if mask.sum() == 0:
        return np.zeros((B, S, D), dtype=np.float32)
    nc = _get_program()
    in_maps = prep_inputs(q, k, v, mask)
    res = run_bass_kernel_spmd(nc, in_maps, list(range(NCORES)))
    return np.concatenate(
        [res.results[c]["out"].astype(np.float32) for c in range(NCORES)], axis=0
    )


# revision 21
# speedup vs baseline: 1.1506x; 1.1506x over previous
"""Trainium2 Bass kernel for batched masked attention.

Problem: q,k,v [16, 2048, 256] f32, mask [16, 2048, 2048] int32.
  scores = (q @ k^T) / 16
  scores = where(mask == 0, 0.0, scores)      # NOT -inf
  att    = softmax(scores, axis=-1)
  att    = 0 if mask.sum() == 0 (handled host-side)
  out    = att @ v

Sharding: batch dim across 8 NeuronCores (2 batches per core); each core
computes full attention for its batches independently; host gathers.

The host pre-arranges inputs into the exact on-chip layouts (all free — the
kernel owns its input contract):
  qh/ql/kh/kl : [BPC, 128, 2, S] fp8e4 — hi/lo split of q/k, head-dim on
          partitions (transposed), the middle dim is the DoubleRow K-group
  vp    : [BPC, 128, S/128, D+2] bf16 — v tiles with two ones columns; in the
          output matmul the ones column accumulates the softmax denominator Z
  mask8 : [BPC, 4, 128, S/128, 512] u8 — mask transposed (key-major) and cast
          to u8, pre-tiled per 512-query chunk
Everything is computed in the transposed score domain so no on-chip
transposes are needed at all; per 512-query chunk:
  mm1 (PE, fp8 DoubleRow, 3-term hi/lo compensation — q@k to ~bf16 accuracy
      at 0.75x the bf16 instruction cost; each DR matmul contracts the full
      K=256 in one shot):
        sT[128 key, 512 qry] = kh.T qh + kh.T ql + kl.T qh
  DVE in-place:            sT = (sT * 1/16) * mask8           (u8 mask)
  ACT:                     attT = exp(sT)  PSUM->SBUF, bf16
  mm2 (PE, bf16):          out[128 qry, 258] += attT.T @ v'   (16 accums)
  DVE: 1/Z + scale-copy -> out tile (bf16) -> DMA; host upcasts to f32
mm2 for chunk ic-1 is emitted after mm1 of chunk ic (software pipelining) so
the PE never idles on the DVE/ACT epilogue; batch-level loads ride the
gpsimd SWDGE ring to keep the sync ring free for mask/out streaming.
"""

import sys

if "/opt/trn_rl_repo" not in sys.path:
    sys.path.insert(0, "/opt/trn_rl_repo")

from contextlib import ExitStack

import numpy as np
import ml_dtypes

import concourse.mybir as mybir
import concourse.tile as tile
from concourse import bacc
from concourse.bass_utils import run_bass_kernel_spmd

B, S, D = 16, 2048, 256
NCORES = 8
BPC = B // NCORES  # batches per core
P = 128
QT = S // P        # 16 key blocks of 128
IC = S // 512      # 4 query chunks of 512
KC = D // P        # 2 contraction chunks of 128 (the DoubleRow K-groups)
SCALE = 1.0 / 16.0  # 1/sqrt(D)

F32 = mybir.dt.float32
BF16 = mybir.dt.bfloat16
FP8 = mybir.dt.float8e4
U8 = mybir.dt.uint8
DR = mybir.MatmulPerfMode.DoubleRow


PIPE_DEPTH = 1


def build_program(reps=1):
    nc = bacc.Bacc("TRN2", target_bir_lowering=False, debug=False)
    # qq/kk pack the hi/lo fp8 pair in one tensor: [., P, 2(hi/lo), KC, S]
    qqd = nc.dram_tensor("qq", [BPC, P, 2, KC, S], FP8, kind="ExternalInput").ap()
    kkd = nc.dram_tensor("kk", [BPC, P, 2, KC, S], FP8, kind="ExternalInput").ap()
    vpd = nc.dram_tensor("vp", [BPC, P, QT, D + 2], BF16, kind="ExternalInput").ap()
    m8d = nc.dram_tensor("mask8", [BPC, IC, P, QT, 512], U8, kind="ExternalInput").ap()
    out = nc.dram_tensor("out", [BPC, S, D], BF16, kind="ExternalOutput").ap()

    with tile.TileContext(nc) as tc, ExitStack() as ctx:
        # bufs=1 on the batch-level inputs: the WAR dependency on the single
        # buffer is what gates the next batch's prefetch DMAs (they would
        # otherwise jump into the startup window and starve the critical
        # loads — the DMA pool is strictly arrival-ordered).
        kk_pool = ctx.enter_context(tc.tile_pool(name="kk", bufs=1))
        qq_pool = ctx.enter_context(tc.tile_pool(name="qq", bufs=1))
        vp_pool = ctx.enter_context(tc.tile_pool(name="vp", bufs=1))
        mask_pool = ctx.enter_context(tc.tile_pool(name="maskp", bufs=3))
        att_pool = ctx.enter_context(tc.tile_pool(name="att", bufs=3))
        osb_pool = ctx.enter_context(tc.tile_pool(name="osb", bufs=4))
        rec_pool = ctx.enter_context(tc.tile_pool(name="rec", bufs=4))
        # ps_s tiles span 2 PSUM banks (a PAIR of key blocks) so one DVE op
        # and one ACT exp cover 1024 columns, halving their per-op overhead
        ps_s = ctx.enter_context(tc.tile_pool(name="ps_s", bufs=3, space="PSUM"))
        ps_out = ctx.enter_context(tc.tile_pool(name="ps_out", bufs=2, space="PSUM"))

        def build_inputs(b, first=False):
            kk = kk_pool.tile([P, 2, KC, S], FP8, tag="kk")
            qq = qq_pool.tile([P, 2, KC, S], FP8, tag="qq")
            vp = vp_pool.tile([P, QT, D + 2], BF16, tag="vp")
            mt0 = None
            if first:
                # Startup-critical: everything on the sync (HWDGE) queue in
                # exact dependency order (the DMA pool is strictly arrival-
                # ordered, and HWDGE generation is ~1.2us per dma_start, so
                # few DMAs, ordered by first use): q/k for chunk 0, first
                # mask piece, rest of k, rest of q, remaining mask pieces,
                # then v (first needed by mm2 ~15us in).
                mt0 = mask_pool.tile([P, QT, 512], U8, tag="maskt")
                nc.sync.dma_start(qq[:, :, :, :512], qqd[b][:, :, :, :512])
                nc.sync.dma_start(kk[:, :, :, :1024], kkd[b][:, :, :, :1024])
                nc.sync.dma_start(mt0[:, :4, :], m8d[b, 0, :, :4, :])
                nc.sync.dma_start(kk[:, :, :, 1024:], kkd[b][:, :, :, 1024:])
                for g4 in range(1, 4):
                    nc.sync.dma_start(
                        mt0[:, g4 * 4 : (g4 + 1) * 4, :],
                        m8d[b, 0, :, g4 * 4 : (g4 + 1) * 4, :],
                    )
                nc.sync.dma_start(qq[:, :, :, 512:], qqd[b][:, :, :, 512:])
                nc.sync.dma_start(vp[:], vpd[b])
            else:
                # Steady-state prefetch: few big transfers on the gpsimd
                # SWDGE ring (SWDGE generation is ~1us per dma_start). The
                # WAR dependency on the single-buffered tiles delays these
                # to the previous batch's last use — just-in-time.
                nc.gpsimd.dma_start(kk[:], kkd[b])
                nc.gpsimd.dma_start(qq[:], qqd[b])
                nc.gpsimd.dma_start(vp[:], vpd[b])
            return (kk, qq), vp, mt0

        def mm1_group(b, ic, g, kk, qq, mt, att):
            """scoresT + mask + exp for key blocks 4g..4g+3 of query chunk ic."""
            qsl = slice(ic * 512, (ic + 1) * 512)
            for jp in range(2 * g, 2 * g + 2):  # pairs of key blocks
                ps = ps_s.tile([P, 1024], F32, tag="score")
                for half in range(2):
                    jb = 2 * jp + half
                    ksl = slice(jb * P, (jb + 1) * P)
                    po = ps[:, half * 512 : (half + 1) * 512]
                    # 3-term compensated fp8 product; each DoubleRow matmul
                    # contracts both 128-dim groups (full K=256) at once
                    nc.tensor.matmul(
                        po, lhsT=kk[:, 0, :, ksl], rhs=qq[:, 0, :, qsl],
                        start=True, stop=False, perf_mode=DR,
                    )
                    nc.tensor.matmul(
                        po, lhsT=kk[:, 0, :, ksl], rhs=qq[:, 1, :, qsl],
                        start=False, stop=False, perf_mode=DR,
                    )
                    nc.tensor.matmul(
                        po, lhsT=kk[:, 1, :, ksl], rhs=qq[:, 0, :, qsl],
                        start=False, stop=True, perf_mode=DR,
                    )
                nc.vector.scalar_tensor_tensor(
                    out=ps[:],
                    in0=ps[:],
                    scalar=SCALE,
                    in1=mt[:, 2 * jp : 2 * jp + 2, :],
                    op0=mybir.AluOpType.mult,
                    op1=mybir.AluOpType.mult,
                )
                nc.scalar.activation(
                    att[:, 2 * jp : 2 * jp + 2, :],
                    ps[:],
                    mybir.ActivationFunctionType.Exp,
                )

        def mm2_group(b, ic, att, vp, iq):
            """att.T @ v' + normalize + store for query tile iq of chunk ic."""
            po = ps_out.tile([P, D + 2], F32, tag="ps_out")
            for jb in range(QT):
                nc.tensor.matmul(
                    po[:],
                    lhsT=att[:, jb, iq * P : (iq + 1) * P],
                    rhs=vp[:, jb, :],
                    start=(jb == 0),
                    stop=(jb == QT - 1),
                )
            rec = rec_pool.tile([P, 1], F32, tag="rec")
            nc.vector.reciprocal(rec[:], po[:, D : D + 1])
            osb = osb_pool.tile([P, D], BF16, tag="osb")
            nc.scalar.activation(
                osb[:],
                po[:, :D],
                mybir.ActivationFunctionType.Copy,
                scale=rec[:],
            )
            it = ic * 4 + iq
            nc.sync.dma_start(out[b, it * P : (it + 1) * P, :], osb[:])

        # Software-pipelined emission: mm2 groups for chunk ic-1 interleave
        # with mm1 groups for chunk ic, so the PE never waits on the DVE/ACT
        # epilogue; next batch's loads are emitted mid-batch for prefetch.
        batches = [b for _ in range(reps) for b in range(BPC)]
        # PE warm-up: ~4us of dummy matmuls during the initial DMA wait so
        # the HAM clock gate is at 2.4 GHz when real work arrives. memset on
        # DVE (idle at t=0; a Pool memset would add ~1.2us of launch latency
        # ahead of the first warmup matmul); a tiny first matmul starts the
        # ramp while the big memset is still in flight.
        warm0 = mask_pool.tile([P, 64], F32, tag="warm0")
        warm = mask_pool.tile([P, 512], F32, tag="warm")
        nc.vector.memset(warm0[:], 0.0)
        nc.vector.memset(warm[:], 0.0)
        # preload the ACT Exp table with a dummy activation so the first real
        # exp doesn't pay the lazy 1.3us ACT_TABLE_LOAD on the critical path
        wdummy = mask_pool.tile([P, 1], F32, tag="wdummy")
        nc.scalar.activation(
            wdummy[:], warm0[:, :1], mybir.ActivationFunctionType.Exp
        )
        wp0 = ps_out.tile([P, 512], F32, tag="ps_out")
        nc.tensor.matmul(
            wp0[:64, :64], lhsT=warm0[:], rhs=warm0[:], start=True, stop=True
        )
        for i in range(2):
            wp = ps_out.tile([P, 512], F32, tag="ps_out")
            nc.tensor.matmul(
                wp[:], lhsT=warm[:, :P], rhs=warm[:], start=True, stop=True
            )
        inputs = {0: build_inputs(batches[0], first=True)}
        # 2-deep pending queue: mm2 for chunk ic-2 interleaves with mm1 of
        # chunk ic, giving the DVE->ACT chain a full extra chunk of latency
        # headroom before the PE's in-order queue demands the att tile.
        from collections import deque

        pending = deque()
        for idx, b in enumerate(batches):
            (kk, qq), vp, mt0 = inputs.pop(idx)
            for ic in range(IC):
                if mt0 is not None and ic == 0:
                    mt = mt0
                else:
                    mt = mask_pool.tile([P, QT, 512], U8, tag="maskt")
                    nc.sync.dma_start(mt[:], m8d[b, ic])
                att = att_pool.tile([P, QT, 512], BF16, tag="att")
                top = pending.popleft() if len(pending) >= PIPE_DEPTH else None
                # At a batch boundary the next batch's q/k arrive just-in-time
                # (WAR-gated prefetch), so emit the pending mm2 group first —
                # mm1 at the PE queue head would block it.
                mm2_first = idx > 0 and ic == 0
                for g in range(4):
                    if top is not None and mm2_first:
                        mm2_group(*top, iq=g)
                    mm1_group(b, ic, g, kk, qq, mt, att)
                    if top is not None and not mm2_first:
                        mm2_group(*top, iq=g)
                if ic == 1 and idx + 1 < len(batches):
                    inputs[idx + 1] = build_inputs(batches[idx + 1])
                pending.append((b, ic, att, vp))
        while pending:
            top = pending.popleft()
            for g in range(4):
                mm2_group(*top, iq=g)

    nc.compile()
    return nc


def prep_inputs(q, k, v, mask):
    """Host-side layout prep; returns per-core in_maps."""
    q = np.asarray(q, dtype=np.float32)
    k = np.asarray(k, dtype=np.float32)
    v = np.asarray(v, dtype=np.float32)
    FP8NP = ml_dtypes.float8_e4m3
    BF16NP = ml_dtypes.bfloat16
    # [B, S, D] -> [B, P, KC, S]  (transposed, head-dim on partitions)
    qt = np.ascontiguousarray(
        q.transpose(0, 2, 1).reshape(B, KC, P, S).transpose(0, 2, 1, 3)
    )
    kt = np.ascontiguousarray(
        k.transpose(0, 2, 1).reshape(B, KC, P, S).transpose(0, 2, 1, 3)
    )
    # hi/lo fp8 split: x = hi + lo to ~0.1% relative accuracy, packed as
    # [B, P, 2(hi/lo), KC, S]
    qh = qt.astype(FP8NP)
    ql = (qt - qh.astype(np.float32)).astype(FP8NP)
    kh = kt.astype(FP8NP)
    kl = (kt - kh.astype(np.float32)).astype(FP8NP)
    qq = np.ascontiguousarray(np.stack([qh, ql], axis=2))
    kk = np.ascontiguousarray(np.stack([kh, kl], axis=2))
    # [B, S, D] -> [B, P, QT, D+2] with ones in the last two columns
    vp = np.ones((B, P, QT, D + 2), dtype=BF16NP)
    vp[..., :D] = v.reshape(B, QT, P, D).transpose(0, 2, 1, 3).astype(BF16NP)
    # mask [B, S(query), S(key)] -> u8 tiles [B, IC, P(key), QT, 512(query)]
    m8 = np.ascontiguousarray(
        (np.asarray(mask) != 0)
        .astype(np.uint8)
        .reshape(B, IC, 512, QT, P)
        .transpose(0, 1, 4, 3, 2)
    )
    return [
        {
            "qq": qq[c * BPC : (c + 1) * BPC],
            "kk": kk[c * BPC : (c + 1) * BPC],
            "vp": vp[c * BPC : (c + 1) * BPC],
            "mask8": m8[c * BPC : (c + 1) * BPC],
        }
        for c in range(NCORES)
    ]


_NC_CACHE = None


def _get_program():
    global _NC_CACHE
    if _NC_CACHE is None:
        _NC_CACHE = build_program()
    return _NC_CACHE


def kernel(q, k, v, mask):
    mask = np.asarray(mask)
    if mask.sum() == 0:
        return np.zeros((B, S, D), dtype=np.float32)
    nc = _get_program()
    in_maps = prep_inputs(q, k, v, mask)
    res = run_bass_kernel_spmd(nc, in_maps, list(range(NCORES)))
    return np.concatenate(
        [res.results[c]["out"].astype(np.float32) for c in range(NCORES)], axis=0
    )
